# revision 1
# baseline (speedup 1.0000x reference)
"""Causal self-attention (B=2, T=2048, C=1024, H=16) on 8 TRN2 NeuronCores.

Sharding: data-parallel over batch x tensor-parallel over heads.
Core c handles batch c//4 and the 4 heads (c%4)*4 .. (c%4)*4+3:
  - QKV projection restricted to its heads' columns of W_attn
  - per-head causal attention (scores kept transposed: ST[j, i])
  - softmax denominator obtained by augmenting V with a ones column,
    so P@V and the row sums come from the same matmuls
  - row-parallel output projection with its heads' rows of W_proj
Host sums the 4 partial projections per batch and adds b_proj.

Matmuls run in float32r (tf32-like, ~1.5e-4 rel err, 4x the fp32 rate).
"""
import sys
sys.path.insert(0, '/opt/trn_rl_repo')

from contextlib import ExitStack

import numpy as np

import concourse.bass as bass
import concourse.tile as tile
from concourse import mybir

B, T, C, H, HD = 2, 2048, 1024, 16, 64
N_CORES = 8
HPC = H // (N_CORES // B)     # heads per core = 4
CPH = HPC * HD                # channel slice per core = 256

f32 = mybir.dt.float32
f32r = mybir.dt.float32r
bf16 = mybir.dt.bfloat16
AF = mybir.ActivationFunctionType

# ---------------------------------------------------------------------------
# Workaround for this container's walrus codegen, which rejects instructions
# carrying more than one sync-wait command ("Too many sync wait commands").
# After Tile scheduling, hoist excess waits onto same-engine NoOps inserted
# immediately before the owning instruction (engine streams are sequential,
# so this preserves semantics exactly).
# ---------------------------------------------------------------------------
import concourse.tile as tile_mod
from bass_rust import ScopedClock, SyncInfo

MAX_WAITS = 1


def _drain_and_barrier(self, tick_clock, wait_clock):
    nc = self.nc
    drain_inst = nc.sync.drain()
    wait_clock.add_sem_waits(
        drain_inst.ins, ScopedClock({None: tick_clock.global_clock})
    )
    si = drain_inst.ins.sync_info
    if si is not None and len(si.on_wait) > MAX_WAITS:
        waits = list(si.on_wait)
        drain_inst.ins.sync_info = SyncInfo(
            on_wait=waits[:MAX_WAITS], on_update=list(si.on_update)
        )
        for k in range(MAX_WAITS, len(waits), MAX_WAITS):
            nop = nc.sync.nop(nofuse=True)
            nop.ins.sync_info = SyncInfo(on_wait=waits[k:k + MAX_WAITS], on_update=[])
    nc.all_engine_barrier()
    assert self.sems is not None
    popped = nc._tile_sem_poison_stack.pop()
    assert popped is self._sem_poison
    nc.clear_and_free_semaphores(list(self.sems.allocated().values()))
    nc.all_engine_barrier()


tile_mod.TileContext._drain_and_barrier = _drain_and_barrier

_split_counter = [0]


def split_excess_waits(nc, max_waits=MAX_WAITS):
    n_split = 0
    for f in nc.m.functions:
        for bb in f.blocks:
            il = bb.instructions
            out = []
            for ins in il:
                si = ins.sync_info
                if si is not None and len(si.on_wait) > max_waits:
                    waits = list(si.on_wait)
                    extra = waits[:-max_waits]
                    for k in range(0, len(extra), max_waits):
                        _split_counter[0] += 1
                        nop = mybir.InstNoOp(
                            name=f"wsplit-{_split_counter[0]}", ins=[], outs=[]
                        )
                        nop.engine = ins.engine
                        nop.sync_info = SyncInfo(
                            on_wait=extra[k:k + max_waits], on_update=[]
                        )
                        out.append(nop)
                    ins.sync_info = SyncInfo(
                        on_wait=waits[-max_waits:], on_update=list(si.on_update)
                    )
                    n_split += 1
                out.append(ins)
            if len(out) != len(il):
                il[:] = out
    return n_split


# ---------------------------------------------------------------------------
# Program builder
# ---------------------------------------------------------------------------
def build_program(reps=1, split_waits=True, phases="abcd"):
    nc = bass.Bass("TRN2", target_bir_lowering=False, debug=False)

    x_d = nc.dram_tensor("x", [T, C], bf16, kind="ExternalInput")
    wqkv_d = nc.dram_tensor("wqkv", [C, 3 * CPH], bf16, kind="ExternalInput")
    bqk_d = nc.dram_tensor("bqk", [128, 4], f32, kind="ExternalInput")
    bv_d = nc.dram_tensor("bv", [128, CPH], f32, kind="ExternalInput")
    wp_d = nc.dram_tensor("wp", [CPH, C], f32, kind="ExternalInput")
    masks_d = nc.dram_tensor("masks", [4, 128, 512], bf16, kind="ExternalInput")
    ident_d = nc.dram_tensor("ident", [128, 128], bf16, kind="ExternalInput")
    y_d = nc.dram_tensor("y", [T, C], f32, kind="ExternalOutput")

    NT = T // 128    # 16 t-blocks
    NCB = C // 128   # 8 c-blocks
    NI = T // 512    # 4 i-chunks

    with tile.TileContext(nc) as tc:
        with ExitStack() as ctx:
            const = ctx.enter_context(tc.tile_pool(name="const", bufs=1))
            ident_t = const.tile([128, 128], bf16, tag="ident")
            nc.sync.dma_start(ident_t[:], ident_d.ap())
            masks_t = []
            for k in range(4):
                mt = const.tile([128, 512], bf16, tag=f"mask{k}", name=f"mask{k}")
                nc.sync.dma_start(mt[:], masks_d.ap()[k])
                masks_t.append(mt)
            bqk_t = const.tile([128, 4], f32, tag="bqk")
            nc.sync.dma_start(bqk_t[:], bqk_d.ap())
            bv_t = const.tile([128, CPH], f32, tag="bv")
            nc.sync.dma_start(bv_t[:], bv_d.ap())
            ones4_t = const.tile([128, 4], f32, tag="ones4")
            nc.gpsimd.memset(ones4_t[:], 1.0)
            tones_f = const.tile([1, 64], f32, tag="tones_f")
            nc.gpsimd.memset(tones_f[:], 1.0)
            tones_t = const.tile([1, 64], f32r, tag="tones")
            nc.vector.tensor_copy(tones_t[:], tones_f[:])

            def body():
                with ExitStack() as c2:
                    # ---- persistent intermediates --------------------------------
                    qk_p = c2.enter_context(tc.tile_pool(name="qk", bufs=1))
                    va_p = c2.enter_context(tc.tile_pool(name="va", bufs=1))
                    yt_p = c2.enter_context(tc.tile_pool(name="yt", bufs=1))
                    # qkt[0..1]: Q^T two heads per tile; qkt[2..3]: K^T
                    qkt = [qk_p.tile([128, T], bf16, tag=f"qkt{m}", name=f"qkt{m}") for m in range(4)]
                    # V augmented with a ones column per head: [128, 4*(64+1)]
                    vaug = [va_p.tile([128, 4 * 65], bf16, tag=f"va{tb}", name=f"va{tb}")
                            for tb in range(NT)]
                    # normalized Y^T, two heads stacked per tile
                    yts = [yt_p.tile([128, T], f32r, tag=f"yts{k}", name=f"yts{k}") for k in range(2)]
                    wpt = [yt_p.tile([128, C], f32r, tag=f"wp{kb}", name=f"wpt{kb}")
                           for kb in range(2)]
                    wpf_tiles = [yt_p.tile([128, C], f32, tag=f"wpf{kb}",
                                           name=f"wpf{kb}") for kb in range(2)]

                    # ---- phase A/B: x^T via PE transpose (bf16), bf16 QKV --------
                    with ExitStack() as c3:
                        xt_p = c3.enter_context(tc.tile_pool(name="xt", bufs=1))
                        xn_p = c3.enter_context(tc.tile_pool(name="xn", bufs=4))
                        w_p = c3.enter_context(tc.tile_pool(name="w", bufs=1))
                        tps = c3.enter_context(
                            tc.tile_pool(name="tps", bufs=2, space="PSUM"))
                        qkps = c3.enter_context(
                            tc.tile_pool(name="qkps", bufs=2, space="PSUM"))
                        vps = c3.enter_context(
                            tc.tile_pool(name="vps", bufs=2, space="PSUM"))

                        xt = [xt_p.tile([128, T], bf16, tag=f"xt{cb}", name=f"xt{cb}")
                              for cb in range(NCB)]
                        wt = [w_p.tile([128, 3 * CPH], bf16, tag=f"wt{cb}", name=f"wt{cb}")
                              for cb in range(NCB)]
                        for tb in range(NT):
                            xn = xn_p.tile([128, C], bf16)
                            nc.sync.dma_start(
                                xn[:], x_d.ap()[tb * 128:(tb + 1) * 128, :])
                            if tb == 4:  # x head-start; weights fill in behind
                                for cb in range(NCB):
                                    nc.sync.dma_start(
                                        wt[cb][:],
                                        wqkv_d.ap()[cb * 128:(cb + 1) * 128, :])
                            for cb in range(NCB):
                                tp = tps.tile([128, 128], bf16)
                                nc.tensor.transpose(
                                    tp[:], xn[:, cb * 128:(cb + 1) * 128],
                                    ident_t[:])
                                nc.vector.tensor_copy(
                                    xt[cb][:, tb * 128:(tb + 1) * 128], tp[:])
                        for kb in range(2):
                            nc.sync.dma_start(
                                wpf_tiles[kb][:],
                                wp_d.ap()[kb * 128:(kb + 1) * 128, :])
                            nc.vector.tensor_copy(wpt[kb][:], wpf_tiles[kb][:])

                        # Q^T / K^T: out[m*128 cols of qkv, t]
                        for m in range(4):
                            for ti in range(NI):
                                ps = qkps.tile([128, 512], f32)
                                for cb in range(NCB):
                                    nc.tensor.matmul(
                                        ps[:],
                                        lhsT=wt[cb][:, m * 128:(m + 1) * 128],
                                        rhs=xt[cb][:, ti * 512:(ti + 1) * 512],
                                        start=(cb == 0), stop=(cb == NCB - 1))
                                nc.vector.tensor_scalar_add(
                                    qkt[m][:, ti * 512:(ti + 1) * 512], ps[:],
                                    bqk_t[:, m:m + 1])

                        # V natural [t, 256] -> vaug tiles with ones columns
                        for tb in range(NT):
                            ps = vps.tile([128, CPH], f32)
                            for cb in range(NCB):
                                nc.tensor.matmul(
                                    ps[:],
                                    lhsT=xt[cb][:, tb * 128:(tb + 1) * 128],
                                    rhs=wt[cb][:, 2 * CPH:3 * CPH],
                                    start=(cb == 0), stop=(cb == NCB - 1))
                            vv = vaug[tb][:].rearrange("p (h e) -> p h e", e=65)
                            nc.vector.tensor_add(
                                vv[:, :, 0:64],
                                ps[:].rearrange("p (h d) -> p h d", d=64),
                                bv_t[:].rearrange("p (h d) -> p h d", d=64))
                            nc.vector.tensor_copy(
                                vv[:, :, 64:65],
                                ones4_t[:].rearrange("p (h e) -> p h e", e=1))

                    # ---- phase C: causal attention per head ----------------------
                    if "c" not in phases:
                        for k2 in range(2):
                            nc.vector.memset(yts[k2][:].bitcast(f32), 0.0)
                    if "c" in phases:
                        with ExitStack() as c3:
                          sps = c3.enter_context(
                              tc.tile_pool(name="sps", bufs=3, space="PSUM"))
                          yps = c3.enter_context(
                              tc.tile_pool(name="yps", bufs=2, space="PSUM"))
                          bps = c3.enter_context(
                              tc.tile_pool(name="bps", bufs=1, space="PSUM"))
                          pps = c3.enter_context(
                              tc.tile_pool(name="pps", bufs=2, space="PSUM"))
                          op = c3.enter_context(tc.tile_pool(name="op", bufs=3))
                          ep = c3.enter_context(tc.tile_pool(name="ep", bufs=6))
                          bp = c3.enter_context(tc.tile_pool(name="bp", bufs=2))
                          rp = c3.enter_context(tc.tile_pool(name="rp", bufs=2))

                          for ci in range(NI):
                              for h in range(HPC):
                                  prow = slice((h % 2) * 64, (h % 2) * 64 + 64)
                                  qt_t = qkt[h // 2]
                                  kt_t = qkt[2 + h // 2]
                                  isl = slice(ci * 512, ci * 512 + 512)
                                  jmax = 4 * ci + 3
                                  yt = yps.tile([128, 512], f32)
                                  for bj in range(jmax + 1):
                                      jsl = slice(bj * 128, bj * 128 + 128)
                                      k = bj - 4 * ci
                                      lo = max(k, 0) * 128  # first valid i column
                                      st = sps.tile([128, 512], f32)
                                      # ST[j, i] = K[j,:] . Q[i,:]
                                      nc.tensor.matmul(
                                          st[:, lo:], lhsT=kt_t[prow, jsl],
                                          rhs=qt_t[prow, isl][:, lo:],
                                          start=True, stop=True)
                                      es = ep.tile([128, 512], bf16, tag="es")
                                      nc.scalar.activation(
                                          es[:, lo:], st[:, lo:], AF.Exp,
                                          scale=0.125)
                                      if k >= 0:  # diagonal band: causal mask
                                          es2 = ep.tile([128, 512], bf16, tag="es2")
                                          nc.vector.tensor_mul(
                                              es2[:, lo:], es[:, lo:],
                                              masks_t[k][:, lo:])
                                          es = es2
                                      # accumulate [Y^T ; rowsum] = V_aug^T @ expST
                                      nc.tensor.matmul(
                                          yt[0:65, lo:],
                                          lhsT=vaug[bj][:, h * 65:(h + 1) * 65],
                                          rhs=es[:, lo:],
                                          start=(bj == 0), stop=(bj == jmax))
                                  rc = rp.tile([1, 512], f32r, tag="rc")
                                  with nc.allow_low_precision(
                                          reason="f32r operand for bcast matmul"):
                                      nc.vector.reciprocal(rc[:], yt[64:65, :])
                                  bc = bps.tile([64, 512], f32)
                                  nc.tensor.matmul(bc[:], lhsT=tones_t[:], rhs=rc[:],
                                                   start=True, stop=True)
                                  bs = bp.tile([64, 512], f32, tag="bs")
                                  nc.vector.tensor_copy(bs[:], bc[:])
                                  nc.vector.tensor_mul(
                                      yts[h // 2][prow, isl], yt[0:64, :], bs[:])
                              if "d" in phases:
                                  for tb in range(4 * ci, 4 * ci + 4):
                                      for nn_ in range(2):
                                          ps = pps.tile([128, 512], f32)
                                          for kb in range(2):
                                              nc.tensor.matmul(
                                                  ps[:],
                                                  lhsT=yts[kb][:, tb * 128:(tb + 1) * 128],
                                                  rhs=wpt[kb][:, nn_ * 512:(nn_ + 1) * 512],
                                                  start=(kb == 0), stop=(kb == 1))
                                          ob = op.tile([128, 512], f32)
                                          nc.vector.tensor_copy(ob[:], ps[:])
                                          nc.sync.dma_start(
                                              y_d.ap()[tb * 128:(tb + 1) * 128,
                                                       nn_ * 512:(nn_ + 1) * 512],
                                              ob[:])

            if reps == 1:
                body()
            else:
                with tc.For_i(0, reps, 1, hint_engines=(
                        mybir.EngineType.PE, mybir.EngineType.Activation,
                        mybir.EngineType.DVE, mybir.EngineType.SP,
                        mybir.EngineType.Pool)):
                    body()

    if split_waits:
        split_excess_waits(nc)
    return nc


# ---------------------------------------------------------------------------
# Cached PJRT runner (fork of concourse.bass2jax.run_bass_via_pjrt that keeps
# the jitted executable so repeat kernel() calls don't recompile)
# ---------------------------------------------------------------------------
_RUNNERS = {}


def _make_pjrt(nc, donate=True, tag="main"):
    import jax
    from jax.sharding import Mesh, PartitionSpec
    from jax.experimental.shard_map import shard_map
    from concourse import bass2jax as b2j

    b2j.install_neuronx_cc_hook()

    partition_name = (
        nc.partition_id_tensor.name if nc.partition_id_tensor else None
    )
    in_names, out_names, out_avals, zero_outs = [], [], [], []
    for alloc in nc.m.functions[0].allocations:
        if not isinstance(alloc, mybir.MemoryLocationSet):
            continue
        name = alloc.memorylocations[0].name
        if alloc.kind == "ExternalInput":
            if name != partition_name:
                in_names.append(name)
        elif alloc.kind == "ExternalOutput":
            out_names.append(name)
            shape = tuple(alloc.tensor_shape)
            dtype = mybir.dt.np(alloc.dtype)
            out_avals.append(jax.core.ShapedArray(shape, dtype))
            zero_outs.append(np.zeros(shape, dtype))
    n_params = len(in_names)
    n_outs = len(out_avals)
    all_names = in_names + out_names
    if partition_name is not None:
        all_names = all_names + [partition_name]
    donate_idx = tuple(range(n_params, n_params + n_outs))

    def _body(*args):
        operands = list(args)
        if partition_name is not None:
            operands.append(b2j.partition_id_tensor())
        outs = b2j._bass_exec_p.bind(
            *operands,
            out_avals=tuple(out_avals),
            in_names=tuple(all_names),
            out_names=tuple(out_names),
            lowering_input_output_aliases=(),
            sim_require_finite=True,
            sim_require_nnan=True,
            nc=nc,
        )
        return tuple(outs)

    _body.__name__ = f"_body_{tag}"
    _body.__qualname__ = f"_body_{tag}"

    devices = jax.devices()[:N_CORES]
    mesh = Mesh(np.asarray(devices), ("core",))
    in_specs = (PartitionSpec("core"),) * (n_params + n_outs)
    out_specs = (PartitionSpec("core"),) * n_outs
    sharded = jax.jit(
        shard_map(_body, mesh=mesh, in_specs=in_specs, out_specs=out_specs,
                  check_rep=False),
        donate_argnums=donate_idx if donate else (), keep_unused=True)

    def concat_args(in_maps):
        per_core = [[np.asarray(m[name]) for name in in_names] for m in in_maps]
        concat_in = [
            np.concatenate([per_core[c][i] for c in range(N_CORES)], axis=0)
            for i in range(n_params)
        ]
        concat_zeros = [
            np.zeros((N_CORES * z.shape[0], *z.shape[1:]), z.dtype)
            for z in zero_outs
        ]
        return concat_in + concat_zeros

    def run(in_maps):
        out_arrs = sharded(*concat_args(in_maps))
        return [
            {name: np.asarray(out_arrs[i]).reshape(N_CORES, *out_avals[i].shape)[c]
             for i, name in enumerate(out_names)}
            for c in range(N_CORES)
        ]

    info = {
        "sharded": sharded, "concat_args": concat_args, "mesh": mesh,
        "PartitionSpec": PartitionSpec, "jax": jax,
    }
    return run, info


def _get_runner(key, nc):
    if key in _RUNNERS:
        return _RUNNERS[key]
    run, _ = _make_pjrt(nc, donate=True, tag=key)
    _RUNNERS[key] = run
    return run


def get_timed_runner(nc, tag="timed"):
    """No donation, device-resident args: returns (call, dev_args_fn)."""
    run, info = _make_pjrt(nc, donate=False, tag=tag)
    jax = info["jax"]
    sharding = jax.sharding.NamedSharding(
        info["mesh"], info["PartitionSpec"]("core"))

    def prepare(in_maps):
        return [jax.device_put(a, sharding) for a in info["concat_args"](in_maps)]

    def call(dev_args):
        outs = info["sharded"](*dev_args)
        jax.block_until_ready(outs)
        return outs

    return prepare, call


# ---------------------------------------------------------------------------
# Host-side sharding / gathering
# ---------------------------------------------------------------------------
def _make_masks():
    import ml_dtypes
    rj = np.arange(128)[:, None]
    ri = np.arange(512)[None, :]
    return np.stack(
        [((rj + 128 * k) <= ri).astype(ml_dtypes.bfloat16) for k in range(4)],
        axis=0)


def make_in_maps(x, W_attn, b_attn, W_proj):
    import ml_dtypes
    masks = _make_masks()
    ident = np.eye(128, dtype=ml_dtypes.bfloat16)
    in_maps = []
    for c in range(N_CORES):
        b = c // (N_CORES // B)
        g = c % (N_CORES // B)
        cs = slice(CPH * g, CPH * g + CPH)
        wq = W_attn[:, CPH * g:CPH * g + CPH]
        wk = W_attn[:, C + CPH * g:C + CPH * g + CPH]
        wv = W_attn[:, 2 * C + CPH * g:2 * C + CPH * g + CPH]
        wqkv = np.ascontiguousarray(
            np.concatenate([wq, wk, wv], axis=1).astype(ml_dtypes.bfloat16))
        bq = b_attn[CPH * g:CPH * g + CPH]
        bk = b_attn[C + CPH * g:C + CPH * g + CPH]
        bvv = b_attn[2 * C + CPH * g:2 * C + CPH * g + CPH]
        bqk = np.ascontiguousarray(
            np.stack([bq[:128], bq[128:], bk[:128], bk[128:]], axis=1))
        bv_arr = np.ascontiguousarray(
            np.broadcast_to(bvv[None, :], (128, CPH)))
        wp = np.ascontiguousarray(W_proj[cs, :])
        in_maps.append({
            "x": np.ascontiguousarray(x[b].astype(ml_dtypes.bfloat16)),
            "wqkv": wqkv, "bqk": bqk, "bv": bv_arr, "wp": wp,
            "masks": masks, "ident": ident,
        })
    return in_maps


def kernel(x, W_attn, b_attn, W_proj, b_proj):
    x = np.asarray(x, dtype=np.float32)
    W_attn = np.asarray(W_attn, dtype=np.float32)
    b_attn = np.asarray(b_attn, dtype=np.float32)
    W_proj = np.asarray(W_proj, dtype=np.float32)
    b_proj = np.asarray(b_proj, dtype=np.float32)

    if "main" not in _RUNNERS:
        nc = build_program(reps=1)
        run = _get_runner("main", nc)
    else:
        run = _RUNNERS["main"]

    results = run(make_in_maps(x, W_attn, b_attn, W_proj))

    out = np.empty((B, T, C), dtype=np.float32)
    gpb = N_CORES // B
    for b in range(B):
        acc = results[gpb * b]["y"].astype(np.float32).copy()
        for g in range(1, gpb):
            acc += results[gpb * b + g]["y"]
        out[b] = acc + b_proj[None, :]
    return out



# revision 6
# speedup vs baseline: 1.3673x; 1.3673x over previous
"""Causal self-attention (B=2, T=2048, C=1024, H=16) on 8 TRN2 NeuronCores.

Sharding: data-parallel over batch x tensor-parallel over heads.
Core c handles batch c//4 and the 4 heads (c%4)*4 .. (c%4)*4+3:
  - QKV projection restricted to its heads' columns of W_attn
  - per-head causal attention (scores kept transposed: ST[j, i])
  - softmax denominator obtained by augmenting V with a ones column,
    so P@V and the row sums come from the same matmuls
  - row-parallel output projection with its heads' rows of W_proj
Host sums the 4 partial projections per batch and adds b_proj.

v2: x is pre-transposed on the host (no PE transposes / DVE copies),
QKV + attention + out-proj are interleaved per 512-chunk so the Act
engine (exp) starts early and PE stays continuously busy, score tiles
are paired [128,1024] to halve exp instruction count, causal masking
multiplies only the 128x128 diagonal triangle in-place (DVE 4x bf16
mode), elementwise work is spread across Pool+DVE, and the projection
partial outputs return as bf16 (halves output DMA).
"""
import sys
sys.path.insert(0, '/opt/trn_rl_repo')

from contextlib import ExitStack

import numpy as np

import concourse.bass as bass
import concourse.tile as tile
from concourse import mybir

B, T, C, H, HD = 2, 2048, 1024, 16, 64
N_CORES = 8
HPC = H // (N_CORES // B)     # heads per core = 4
CPH = HPC * HD                # channel slice per core = 256

f32 = mybir.dt.float32
f32r = mybir.dt.float32r
bf16 = mybir.dt.bfloat16
AF = mybir.ActivationFunctionType

# ---------------------------------------------------------------------------
# Workaround for this container's walrus codegen, which rejects instructions
# carrying more than one sync-wait command ("Too many sync wait commands").
# After Tile scheduling, hoist excess waits onto same-engine NoOps inserted
# immediately before the owning instruction (engine streams are sequential,
# so this preserves semantics exactly).
# ---------------------------------------------------------------------------
import concourse.tile as tile_mod
from bass_rust import ScopedClock, SyncInfo

MAX_WAITS = 1


def _drain_and_barrier(self, tick_clock, wait_clock):
    nc = self.nc
    drain_inst = nc.sync.drain()
    wait_clock.add_sem_waits(
        drain_inst.ins, ScopedClock({None: tick_clock.global_clock})
    )
    si = drain_inst.ins.sync_info
    if si is not None and len(si.on_wait) > MAX_WAITS:
        waits = list(si.on_wait)
        drain_inst.ins.sync_info = SyncInfo(
            on_wait=waits[:MAX_WAITS], on_update=list(si.on_update)
        )
        for k in range(MAX_WAITS, len(waits), MAX_WAITS):
            nop = nc.sync.nop(nofuse=True)
            nop.ins.sync_info = SyncInfo(on_wait=waits[k:k + MAX_WAITS], on_update=[])
    nc.all_engine_barrier()
    assert self.sems is not None
    popped = nc._tile_sem_poison_stack.pop()
    assert popped is self._sem_poison
    nc.clear_and_free_semaphores(list(self.sems.allocated().values()))
    nc.all_engine_barrier()


tile_mod.TileContext._drain_and_barrier = _drain_and_barrier

_split_counter = [0]


def split_excess_waits(nc, max_waits=MAX_WAITS):
    n_split = 0
    for f in nc.m.functions:
        for bb in f.blocks:
            il = bb.instructions
            out = []
            for ins in il:
                si = ins.sync_info
                if si is not None and len(si.on_wait) > max_waits:
                    waits = list(si.on_wait)
                    extra = waits[:-max_waits]
                    for k in range(0, len(extra), max_waits):
                        _split_counter[0] += 1
                        nop = mybir.InstNoOp(
                            name=f"wsplit-{_split_counter[0]}", ins=[], outs=[]
                        )
                        nop.engine = ins.engine
                        nop.sync_info = SyncInfo(
                            on_wait=extra[k:k + max_waits], on_update=[]
                        )
                        out.append(nop)
                    ins.sync_info = SyncInfo(
                        on_wait=waits[-max_waits:], on_update=list(si.on_update)
                    )
                    n_split += 1
                out.append(ins)
            if len(out) != len(il):
                il[:] = out
    return n_split


# ---------------------------------------------------------------------------
# Program builder
# ---------------------------------------------------------------------------
def build_program(reps=1, split_waits=True, phases="abcd"):
    nc = bass.Bass("TRN2", target_bir_lowering=False, debug=False)

    xt_d = nc.dram_tensor("xt", [C, T], bf16, kind="ExternalInput")
    wqkv_d = nc.dram_tensor("wqkv", [C, 3 * CPH], bf16, kind="ExternalInput")
    bqk_d = nc.dram_tensor("bqk", [128, 4], f32, kind="ExternalInput")
    bv_d = nc.dram_tensor("bv", [128, CPH], f32, kind="ExternalInput")
    wp_d = nc.dram_tensor("wp", [CPH, C], bf16, kind="ExternalInput")
    trimask_d = nc.dram_tensor("trimask", [128, 128], bf16, kind="ExternalInput")
    y_d = nc.dram_tensor("y", [T, C], bf16, kind="ExternalOutput")

    NT = T // 128    # 16 t-blocks
    NCB = C // 128   # 8 c-blocks
    NI = T // 512    # 4 i-chunks

    with tile.TileContext(nc) as tc:
        with ExitStack() as ctx:
            const = ctx.enter_context(tc.tile_pool(name="const", bufs=1))
            trimask_t = const.tile([128, 128], bf16, tag="trimask")
            nc.sync.dma_start(trimask_t[:], trimask_d.ap())
            bqk_t = const.tile([128, 4], f32, tag="bqk")
            nc.sync.dma_start(bqk_t[:], bqk_d.ap())
            bv_t = const.tile([128, CPH], f32, tag="bv")
            nc.sync.dma_start(bv_t[:], bv_d.ap())
            ones4_t = const.tile([128, 4], f32, tag="ones4")
            nc.gpsimd.memset(ones4_t[:], 1.0)
            tones_f = const.tile([1, 64], f32, tag="tones_f")
            nc.gpsimd.memset(tones_f[:], 1.0)
            tones_t = const.tile([1, 64], f32r, tag="tones")
            nc.vector.tensor_copy(tones_t[:], tones_f[:])

            def body():
                with ExitStack() as c2:
                    # ---- persistent SBUF -----------------------------------
                    xw_p = c2.enter_context(tc.tile_pool(name="xw", bufs=1))
                    qk_p = c2.enter_context(tc.tile_pool(name="qk", bufs=1))
                    va_p = c2.enter_context(tc.tile_pool(name="va", bufs=1))
                    yt_p = c2.enter_context(tc.tile_pool(name="yt", bufs=1))
                    xt = [xw_p.tile([128, T], bf16, tag=f"xt{cb}", name=f"xt{cb}")
                          for cb in range(NCB)]
                    wt = [xw_p.tile([128, 3 * CPH], bf16, tag=f"wt{cb}",
                                    name=f"wt{cb}") for cb in range(NCB)]
                    wpt = [xw_p.tile([128, C], bf16, tag=f"wp{kb}",
                                     name=f"wpt{kb}") for kb in range(2)]
                    # qkt[0..1]: Q^T two heads per tile; qkt[2..3]: K^T
                    qkt = [qk_p.tile([128, T], bf16, tag=f"qkt{m}", name=f"qkt{m}")
                           for m in range(4)]
                    # V augmented with a ones column per head: [128, 4*65]
                    vaug = [va_p.tile([128, HPC * 65], bf16, tag=f"va{tb}",
                                      name=f"va{tb}") for tb in range(NT)]
                    # normalized Y^T, two heads stacked per tile
                    yts = [yt_p.tile([128, T], bf16, tag=f"yts{k}", name=f"yts{k}")
                           for k in range(2)]

                    for cb in range(NCB):
                        nc.sync.dma_start(wt[cb][:],
                                          wqkv_d.ap()[cb * 128:(cb + 1) * 128, :])

                    def dma_chunk(ci):
                        csl = slice(ci * 512, ci * 512 + 512)
                        for cb in range(NCB):
                            nc.sync.dma_start(
                                xt[cb][:, csl],
                                xt_d.ap()[cb * 128:(cb + 1) * 128, csl])

                    dma_chunk(0)
                    for kb in range(2):
                        nc.sync.dma_start(wpt[kb][:],
                                          wp_d.ap()[kb * 128:(kb + 1) * 128, :])

                    with ExitStack() as c3:
                        genps = c3.enter_context(
                            tc.tile_pool(name="genps", bufs=2, space="PSUM"))
                        sps = c3.enter_context(
                            tc.tile_pool(name="sps", bufs=2, space="PSUM"))
                        yps = c3.enter_context(
                            tc.tile_pool(name="yps", bufs=2, space="PSUM"))
                        ep = c3.enter_context(tc.tile_pool(name="ep", bufs=16))
                        rp = c3.enter_context(tc.tile_pool(name="rp", bufs=4))
                        op = c3.enter_context(tc.tile_pool(name="op", bufs=4))

                        if "c" not in phases:
                            for k2 in range(2):
                                nc.vector.memset(yts[k2][:].bitcast(f32), 0.0)

                        # ---- emission helpers ------------------------------
                        def qk_mm(ci, m):
                            isl = slice(ci * 512, ci * 512 + 512)
                            ps = genps.tile([128, 512], f32, tag="gen")
                            for cb in range(NCB):
                                nc.tensor.matmul(
                                    ps[:],
                                    lhsT=wt[cb][:, m * 128:(m + 1) * 128],
                                    rhs=xt[cb][:, isl],
                                    start=(cb == 0), stop=(cb == NCB - 1))
                            nc.vector.tensor_scalar_add(
                                qkt[m][:, isl], ps[:], bqk_t[:, m:m + 1])

                        def v_mm(tb):
                            ps = genps.tile([128, 512], f32, tag="gen")
                            for cb in range(NCB):
                                nc.tensor.matmul(
                                    ps[:, 0:CPH],
                                    lhsT=xt[cb][:, tb * 128:(tb + 1) * 128],
                                    rhs=wt[cb][:, 2 * CPH:3 * CPH],
                                    start=(cb == 0), stop=(cb == NCB - 1))
                            vv = vaug[tb][:].rearrange("p (h e) -> p h e", e=65)
                            nc.vector.tensor_add(
                                vv[:, :, 0:64],
                                ps[:, 0:CPH].rearrange("p (h d) -> p h d", d=64),
                                bv_t[:].rearrange("p (h d) -> p h d", d=64))
                            nc.gpsimd.tensor_copy(
                                vv[:, :, 64:65],
                                ones4_t[:].rearrange("p (h e) -> p h e", e=1))

                        def s_pair(ci, h, p):
                            """Scores for j-blocks 2p,2p+1 vs i-chunk ci; exp;
                            diag triangle mask. Returns the es tile."""
                            isl = slice(ci * 512, ci * 512 + 512)
                            prow = slice((h % 2) * 64, (h % 2) * 64 + 64)
                            qt_t = qkt[h // 2]
                            kt_t = qkt[2 + h // 2]
                            st = sps.tile([128, 1024], f32, tag="st")
                            es = ep.tile([128, 1024], bf16, tag="es")
                            for half in range(2):
                                bj = 2 * p + half
                                lo = max(bj - 4 * ci, 0) * 128
                                jsl = slice(bj * 128, bj * 128 + 128)
                                osl = slice(half * 512 + lo, half * 512 + 512)
                                nc.tensor.matmul(
                                    st[:, osl], lhsT=kt_t[prow, jsl],
                                    rhs=qt_t[prow, isl][:, lo:],
                                    start=True, stop=True)
                            if p < 2 * ci:      # both halves full
                                nc.scalar.activation(
                                    es[:], st[:], AF.Exp, scale=0.125)
                            else:               # diagonal pair
                                for half in range(2):
                                    k = 2 * p + half - 4 * ci
                                    lo = k * 128
                                    osl = slice(half * 512 + lo,
                                                half * 512 + 512)
                                    nc.scalar.activation(
                                        es[:, osl], st[:, osl], AF.Exp,
                                        scale=0.125)
                                    msl = slice(half * 512 + lo,
                                                half * 512 + lo + 128)
                                    nc.vector.tensor_mul(
                                        es[:, msl], es[:, msl], trimask_t[:])
                            return es

                        def pv_pair(ci, h, p, yt, es):
                            jmax = 4 * ci + 3
                            for half in range(2):
                                bj = 2 * p + half
                                lo = max(bj - 4 * ci, 0) * 128
                                nc.tensor.matmul(
                                    yt[0:65, lo:],
                                    lhsT=vaug[bj][:, h * 65:(h + 1) * 65],
                                    rhs=es[:, half * 512 + lo:half * 512 + 512],
                                    start=(bj == 0), stop=(bj == jmax))

                        def norm(ci, h, yt):
                            """recip + ones-bcast matmul + scale into yts."""
                            isl = slice(ci * 512, ci * 512 + 512)
                            prow = slice((h % 2) * 64, (h % 2) * 64 + 64)
                            rc = rp.tile([1, 512], f32r, tag="rc")
                            with nc.allow_low_precision(
                                    reason="f32r operand for bcast matmul"):
                                nc.vector.reciprocal(rc[:], yt[64:65, :])
                            bc = genps.tile([128, 512], f32, tag="gen")
                            nc.tensor.matmul(bc[0:64, :], lhsT=tones_t[:],
                                             rhs=rc[:], start=True, stop=True)
                            bs = rp.tile([64, 512], f32, tag="bs")
                            nc.vector.tensor_copy(bs[:], bc[0:64, :])
                            nc.vector.tensor_mul(
                                yts[h // 2][prow, isl], yt[0:64, :], bs[:])

                        def proj(ci):
                            for tb in range(4 * ci, 4 * ci + 4):
                                for nn_ in range(2):
                                    ps = genps.tile([128, 512], f32, tag="gen")
                                    for kb in range(2):
                                        nc.tensor.matmul(
                                            ps[:],
                                            lhsT=yts[kb][:, tb * 128:
                                                         (tb + 1) * 128],
                                            rhs=wpt[kb][:, nn_ * 512:
                                                        (nn_ + 1) * 512],
                                            start=(kb == 0), stop=(kb == 1))
                                    ob = op.tile([128, 512], bf16, tag="ob")
                                    if (tb + nn_) % 2 == 0:
                                        nc.vector.tensor_copy(ob[:], ps[:])
                                    else:
                                        nc.scalar.copy(ob[:], ps[:])
                                    nc.sync.dma_start(
                                        y_d.ap()[tb * 128:(tb + 1) * 128,
                                                 nn_ * 512:(nn_ + 1) * 512],
                                        ob[:])

                        # ---- interleaved emission --------------------------
                        # Per chunk ci: qk matmuls (with previous chunk's last
                        # normalize tucked in), scores h0 zipped with V, then
                        # heads pipelined: scores(h+1) zipped with PV(h);
                        # previous chunk's proj slides in after scores start.
                        pend_norm = None     # (ci, h, yt) awaiting emission
                        for ci in range(NI):
                            npairs = 2 * ci + 2
                            if ci + 1 < NI:
                                dma_chunk(ci + 1)
                            qk_mm(ci, 0)
                            if pend_norm is not None:
                                norm(*pend_norm)
                                pend_norm = None
                            for m in range(1, 4):
                                qk_mm(ci, m)
                            if "c" not in phases:
                                for tb in range(4 * ci, 4 * ci + 4):
                                    v_mm(tb)
                                if "d" in phases:
                                    proj(ci)
                                continue

                            # scores h0 zipped with V matmuls
                            es_cur = []
                            for p in range(npairs):
                                es_cur.append(s_pair(ci, 0, p))
                                if p < 2:
                                    v_mm(4 * ci + 2 * p)
                                    v_mm(4 * ci + 2 * p + 1)
                            if "d" in phases and ci > 0:
                                proj(ci - 1)
                            for h in range(HPC):
                                yt = yps.tile([128, 512], f32, tag="yt")
                                es_next = []
                                for p in range(npairs):
                                    if h + 1 < HPC:
                                        es_next.append(s_pair(ci, h + 1, p))
                                    pv_pair(ci, h, p, yt, es_cur[p])
                                    if p == npairs // 2 and pend_norm is not None:
                                        norm(*pend_norm)
                                        pend_norm = None
                                if pend_norm is not None:
                                    norm(*pend_norm)
                                pend_norm = (ci, h, yt)
                                es_cur = es_next
                        if pend_norm is not None:
                            norm(*pend_norm)
                        if "c" in phases and "d" in phases:
                            proj(NI - 1)

            if reps == 1:
                body()
            else:
                with tc.For_i(0, reps, 1, hint_engines=(
                        mybir.EngineType.PE, mybir.EngineType.Activation,
                        mybir.EngineType.DVE, mybir.EngineType.SP,
                        mybir.EngineType.Pool)):
                    body()

    if split_waits:
        split_excess_waits(nc)
    return nc


# ---------------------------------------------------------------------------
# Cached PJRT runner (fork of concourse.bass2jax.run_bass_via_pjrt that keeps
# the jitted executable so repeat kernel() calls don't recompile)
# ---------------------------------------------------------------------------
_RUNNERS = {}


def _make_pjrt(nc, donate=True, tag="main"):
    import jax
    from jax.sharding import Mesh, PartitionSpec
    from jax.experimental.shard_map import shard_map
    from concourse import bass2jax as b2j

    b2j.install_neuronx_cc_hook()

    partition_name = (
        nc.partition_id_tensor.name if nc.partition_id_tensor else None
    )
    in_names, out_names, out_avals, zero_outs = [], [], [], []
    for alloc in nc.m.functions[0].allocations:
        if not isinstance(alloc, mybir.MemoryLocationSet):
            continue
        name = alloc.memorylocations[0].name
        if alloc.kind == "ExternalInput":
            if name != partition_name:
                in_names.append(name)
        elif alloc.kind == "ExternalOutput":
            out_names.append(name)
            shape = tuple(alloc.tensor_shape)
            dtype = mybir.dt.np(alloc.dtype)
            out_avals.append(jax.core.ShapedArray(shape, dtype))
            zero_outs.append(np.zeros(shape, dtype))
    n_params = len(in_names)
    n_outs = len(out_avals)
    all_names = in_names + out_names
    if partition_name is not None:
        all_names = all_names + [partition_name]
    donate_idx = tuple(range(n_params, n_params + n_outs))

    def _body(*args):
        operands = list(args)
        if partition_name is not None:
            operands.append(b2j.partition_id_tensor())
        outs = b2j._bass_exec_p.bind(
            *operands,
            out_avals=tuple(out_avals),
            in_names=tuple(all_names),
            out_names=tuple(out_names),
            lowering_input_output_aliases=(),
            sim_require_finite=True,
            sim_require_nnan=True,
            nc=nc,
        )
        return tuple(outs)

    _body.__name__ = f"_body_{tag}"
    _body.__qualname__ = f"_body_{tag}"

    devices = jax.devices()[:N_CORES]
    mesh = Mesh(np.asarray(devices), ("core",))
    in_specs = (PartitionSpec("core"),) * (n_params + n_outs)
    out_specs = (PartitionSpec("core"),) * n_outs
    sharded = jax.jit(
        shard_map(_body, mesh=mesh, in_specs=in_specs, out_specs=out_specs,
                  check_rep=False),
        donate_argnums=donate_idx if donate else (), keep_unused=True)

    def concat_args(in_maps):
        per_core = [[np.asarray(m[name]) for name in in_names] for m in in_maps]
        concat_in = [
            np.concatenate([per_core[c][i] for c in range(N_CORES)], axis=0)
            for i in range(n_params)
        ]
        concat_zeros = [
            np.zeros((N_CORES * z.shape[0], *z.shape[1:]), z.dtype)
            for z in zero_outs
        ]
        return concat_in + concat_zeros

    def run(in_maps):
        out_arrs = sharded(*concat_args(in_maps))
        return [
            {name: np.asarray(out_arrs[i]).reshape(N_CORES, *out_avals[i].shape)[c]
             for i, name in enumerate(out_names)}
            for c in range(N_CORES)
        ]

    info = {
        "sharded": sharded, "concat_args": concat_args, "mesh": mesh,
        "PartitionSpec": PartitionSpec, "jax": jax,
    }
    return run, info


def _get_runner(key, nc):
    if key in _RUNNERS:
        return _RUNNERS[key]
    run, _ = _make_pjrt(nc, donate=True, tag=key)
    _RUNNERS[key] = run
    return run


def get_timed_runner(nc, tag="timed"):
    """No donation, device-resident args: returns (call, dev_args_fn)."""
    run, info = _make_pjrt(nc, donate=False, tag=tag)
    jax = info["jax"]
    sharding = jax.sharding.NamedSharding(
        info["mesh"], info["PartitionSpec"]("core"))

    def prepare(in_maps):
        return [jax.device_put(a, sharding) for a in info["concat_args"](in_maps)]

    def call(dev_args):
        outs = info["sharded"](*dev_args)
        jax.block_until_ready(outs)
        return outs

    return prepare, call


# ---------------------------------------------------------------------------
# Host-side sharding / gathering
# ---------------------------------------------------------------------------
def make_in_maps(x, W_attn, b_attn, W_proj):
    import ml_dtypes
    rj = np.arange(128)[:, None]
    ri = np.arange(128)[None, :]
    trimask = (rj <= ri).astype(ml_dtypes.bfloat16)
    in_maps = []
    for c in range(N_CORES):
        b = c // (N_CORES // B)
        g = c % (N_CORES // B)
        cs = slice(CPH * g, CPH * g + CPH)
        wq = W_attn[:, CPH * g:CPH * g + CPH]
        wk = W_attn[:, C + CPH * g:C + CPH * g + CPH]
        wv = W_attn[:, 2 * C + CPH * g:2 * C + CPH * g + CPH]
        wqkv = np.ascontiguousarray(
            np.concatenate([wq, wk, wv], axis=1).astype(ml_dtypes.bfloat16))
        bq = b_attn[CPH * g:CPH * g + CPH]
        bk = b_attn[C + CPH * g:C + CPH * g + CPH]
        bvv = b_attn[2 * C + CPH * g:2 * C + CPH * g + CPH]
        bqk = np.ascontiguousarray(
            np.stack([bq[:128], bq[128:], bk[:128], bk[128:]], axis=1))
        bv_arr = np.ascontiguousarray(
            np.broadcast_to(bvv[None, :], (128, CPH)))
        wp = np.ascontiguousarray(W_proj[cs, :].astype(ml_dtypes.bfloat16))
        in_maps.append({
            "xt": np.ascontiguousarray(x[b].T.astype(ml_dtypes.bfloat16)),
            "wqkv": wqkv, "bqk": bqk, "bv": bv_arr, "wp": wp,
            "trimask": trimask,
        })
    return in_maps


def kernel(x, W_attn, b_attn, W_proj, b_proj):
    x = np.asarray(x, dtype=np.float32)
    W_attn = np.asarray(W_attn, dtype=np.float32)
    b_attn = np.asarray(b_attn, dtype=np.float32)
    W_proj = np.asarray(W_proj, dtype=np.float32)
    b_proj = np.asarray(b_proj, dtype=np.float32)

    if "main" not in _RUNNERS:
        nc = build_program(reps=1)
        run = _get_runner("main", nc)
    else:
        run = _RUNNERS["main"]

    results = run(make_in_maps(x, W_attn, b_attn, W_proj))

    out = np.empty((B, T, C), dtype=np.float32)
    gpb = N_CORES // B
    for b in range(B):
        acc = results[gpb * b]["y"].astype(np.float32)
        for g in range(1, gpb):
            acc = acc + results[gpb * b + g]["y"].astype(np.float32)
        out[b] = acc + b_proj[None, :]
    return out
